# revision 1
# baseline (speedup 1.0000x reference)
"""CRF loss kernel for Trainium2 (8 NeuronCores, data-parallel over batch).

Strategy (per core, batch shard of 64 rows = 32768 positions):
  - emissions gather sum_{b,s} m*E[b,s,tags] via one-hot matmuls on PE:
    E is split exactly as E = bf16(E) + bf16(E - bf16(E)) (17-18 mantissa
    bits kept); both halves go through full-rate bf16 matmuls against a
    bf16 one-hot of the (mask-folded) tags, accumulating in fp32 PSUM.
    Diagonal of the accumulated [T,T] PSUM = emission score.
  - transition score via pair co-occurrence counts C = Hprev^T @ Hcur
    (bf16 one-hots, exact 0/1 counts in fp32 PSUM), then sum(C * T).
  - mask folding: tag + 128*(1-m) pushes masked positions out of iota
    range so their one-hot row is all zero.
  - the two scalar partial sums and the mask count are reduced on-chip
    to a [1,8] vector per core; the 8-way combine + division is host-side.
"""
import sys
import json

for p in ('/opt/trn_rl_repo', '/opt/trn_rl_repo/concourse'):
    if p not in sys.path:
        sys.path.insert(0, p)

import numpy as np

B, S, T = 512, 512, 128
NCORES = 8
BSH = B // NCORES              # 64 batch rows per core
NPOS = BSH * S                 # 32768 positions per core
NTILE = NPOS // 128            # 256 tag-tiles of 128 positions
NBLK = NTILE // 4              # 64 blocks of [128, 4, 128]
# fraction of lo-subtract blocks on DVE (rest on GPSIMD)
LO_DVE_MOD = 3                 # g % LO_DVE_MOD == 0 -> DVE


def _split_waits_json(bir_bytes: bytes, max_waits: int = 1) -> bytes:
    """This walrus build accepts at most ONE sync-wait per instruction;
    hoist extra waits onto single-wait NoOps inserted before the inst."""
    d = json.loads(bir_bytes)
    ctr = 0
    for f in d['functions']:
        for blk in f['blocks']:
            insts = blk.get('instructions')
            if not insts:
                continue
            out = []
            changed = False
            for ins in insts:
                si = ins.get('sync_info')
                if si and len(si.get('on_wait') or []) > max_waits:
                    waits = si['on_wait']
                    for w in waits[:-max_waits]:
                        ctr += 1
                        nop = {'engine': ins['engine'], 'ins': [], 'outs': [],
                               'name': f'wsplit-{ctr}', 'opcode': 'NoOp',
                               'sync_info': {'on_wait': [w], 'on_update': []}}
                        if 'debug' in ins:
                            nop['debug'] = ins['debug']
                        out.append(nop)
                    si['on_wait'] = waits[-max_waits:]
                    changed = True
                out.append(ins)
            if changed:
                blk['instructions'] = out
    return json.dumps(d).encode()


_patched = False


def _install_patch(bass_module):
    global _patched
    if _patched:
        return
    _patched = True
    orig = bass_module.Bass.to_json_bytes

    def patched(self):
        return _split_waits_json(orig(self))

    bass_module.Bass.to_json_bytes = patched


def _build():
    import concourse.bass as bass
    import concourse.mybir as mybir
    import concourse.tile as tile
    from concourse.masks import make_identity
    _install_patch(bass)
    f32 = mybir.dt.float32
    bf16 = mybir.dt.bfloat16
    u16 = mybir.dt.uint16
    i32 = mybir.dt.int32
    Alu = mybir.AluOpType

    nc = bass.Bass()
    em = nc.dram_tensor('em', [NPOS, T], f32, kind='ExternalInput')
    tg = nc.dram_tensor('tg', [NPOS + 1], u16, kind='ExternalInput')
    mk = nc.dram_tensor('mk', [NPOS + 1], u16, kind='ExternalInput')
    tr = nc.dram_tensor('tr', [T, T], f32, kind='ExternalInput')
    out = nc.dram_tensor('out', [1, 8], f32, kind='ExternalOutput')

    with tile.TileContext(nc) as tc:
        with tc.tile_pool(name='per', bufs=1) as per, \
             tc.tile_pool(name='eblk', bufs=3) as eblk, \
             tc.tile_pool(name='hblk', bufs=3) as hblk, \
             tc.tile_pool(name='ps', bufs=1, space='PSUM') as psp:

            # ---- constants ----
            iota_i = per.tile([128, 128], i32)
            nc.gpsimd.iota(iota_i, pattern=[[1, 128]], base=0, channel_multiplier=0)
            iota_b = per.tile([128, 128], bf16)
            nc.vector.tensor_copy(iota_b, iota_i)
            ident = per.tile([128, 128], f32)
            make_identity(nc, ident)
            ones_col = per.tile([128, 1], f32)
            nc.vector.memset(ones_col, 1.0)
            t_sb = per.tile([128, 128], f32)
            nc.sync.dma_start(out=t_sb, in_=tr[:, :])

            # ---- tags / mask (transposed to [pos%128, tile] layout) ----
            tg_cur = per.tile([128, NTILE], u16)
            nc.sync.dma_start_transpose(tg_cur, tg[1:NPOS + 1].rearrange("(a b) -> a b", b=128))
            tg_prev = per.tile([128, NTILE], u16)
            nc.sync.dma_start_transpose(tg_prev, tg[0:NPOS].rearrange("(a b) -> a b", b=128))
            mk_cur = per.tile([128, NTILE], u16)
            nc.sync.dma_start_transpose(mk_cur, mk[1:NPOS + 1].rearrange("(a b) -> a b", b=128))
            mk_prev = per.tile([128, NTILE], u16)
            nc.sync.dma_start_transpose(mk_prev, mk[0:NPOS].rearrange("(a b) -> a b", b=128))

            tgc_f = per.tile([128, NTILE], f32)
            nc.vector.tensor_copy(tgc_f, tg_cur)
            tgp_f = per.tile([128, NTILE], f32)
            nc.vector.tensor_copy(tgp_f, tg_prev)
            mc_f = per.tile([128, NTILE], f32)
            nc.vector.tensor_copy(mc_f, mk_cur)
            mp_f = per.tile([128, NTILE], f32)
            nc.vector.tensor_copy(mp_f, mk_prev)

            # masked cur tags: tg + 128 - 128*m
            tmp = per.tile([128, NTILE], f32)
            nc.vector.tensor_scalar(out=tmp, in0=mc_f, scalar1=-128.0, scalar2=128.0,
                                    op0=Alu.mult, op1=Alu.add)
            mtag_c = per.tile([128, NTILE], f32)
            nc.vector.tensor_add(mtag_c, tgc_f, tmp)

            # pair mask pm = m_cur * m_prev, zeroed at batch-row starts
            pm = per.tile([128, NTILE], f32)
            nc.vector.tensor_mul(pm, mc_f, mp_f)
            rs_i = per.tile([128, NTILE], i32)   # p + 128*(tile%4); ==0 at row starts
            nc.gpsimd.iota(rs_i, pattern=[[0, NTILE // 4], [128, 4]], base=0,
                           channel_multiplier=1)
            rs_f = per.tile([128, NTILE], f32)
            nc.vector.tensor_copy(rs_f, rs_i)
            rs_m = per.tile([128, NTILE], f32)
            nc.vector.tensor_scalar(out=rs_m, in0=rs_f, scalar1=0.0, scalar2=None,
                                    op0=Alu.not_equal)
            nc.vector.tensor_mul(pm, pm, rs_m)

            # masked prev tags: tg_prev + 128 - 128*pm
            nc.vector.tensor_scalar(out=tmp, in0=pm, scalar1=-128.0, scalar2=128.0,
                                    op0=Alu.mult, op1=Alu.add)
            mtag_p = per.tile([128, NTILE], f32)
            nc.vector.tensor_add(mtag_p, tgp_f, tmp)

            # ---- accumulators ----
            ps_emit = psp.tile([128, 128], f32)
            ps_c = psp.tile([128, 128], f32)

            em_r = em.rearrange("(g j p) t -> g p j t", p=128, j=4)

            for g in range(NBLK):
                e_blk = eblk.tile([128, 4, 128], f32, tag='e')
                nc.sync.dma_start(out=e_blk, in_=em_r[g])
                hi_blk = eblk.tile([128, 4, 128], bf16, tag='hi')
                nc.scalar.activation(out=hi_blk, in_=e_blk,
                                     func=mybir.ActivationFunctionType.Copy)
                lo_blk = eblk.tile([128, 4, 128], bf16, tag='lo')
                if g % LO_DVE_MOD == 0:
                    nc.vector.tensor_sub(lo_blk, e_blk, hi_blk)
                else:
                    nc.gpsimd.tensor_sub(lo_blk, e_blk, hi_blk)
                hm = hblk.tile([128, 4, 128], bf16, tag='hm')
                hp = hblk.tile([128, 4, 128], bf16, tag='hp')
                for j in range(4):
                    k = 4 * g + j
                    nc.vector.tensor_scalar(out=hm[:, j, :], in0=iota_b,
                                            scalar1=mtag_c[:, k:k + 1], scalar2=None,
                                            op0=Alu.is_equal)
                    nc.vector.tensor_scalar(out=hp[:, j, :], in0=iota_b,
                                            scalar1=mtag_p[:, k:k + 1], scalar2=None,
                                            op0=Alu.is_equal)
                for j in range(4):
                    first = (g == 0 and j == 0)
                    last = (g == NBLK - 1 and j == 3)
                    nc.tensor.matmul(ps_emit, lhsT=hm[:, j, :], rhs=hi_blk[:, j, :],
                                     start=first, stop=False, skip_group_check=True)
                    nc.tensor.matmul(ps_emit, lhsT=hm[:, j, :], rhs=lo_blk[:, j, :],
                                     start=False, stop=last, skip_group_check=True)
                    nc.tensor.matmul(ps_c, lhsT=hp[:, j, :], rhs=hm[:, j, :],
                                     start=first, stop=last, skip_group_check=True)

            # ---- final reductions ----
            red = per.tile([128, 8], f32)
            nc.vector.memset(red, 0.0)
            scr = per.tile([128, 128], f32)
            nc.vector.tensor_mul(scr, ps_emit, ident)
            nc.vector.tensor_reduce(out=red[:, 0:1], in_=scr,
                                    axis=mybir.AxisListType.X, op=Alu.add)
            scr2 = per.tile([128, 128], f32)
            nc.vector.tensor_mul(scr2, ps_c, t_sb)
            nc.vector.tensor_reduce(out=red[:, 1:2], in_=scr2,
                                    axis=mybir.AxisListType.X, op=Alu.add)
            nc.vector.tensor_reduce(out=red[:, 2:3], in_=mc_f,
                                    axis=mybir.AxisListType.X, op=Alu.add)
            ps_fin = psp.tile([1, 8], f32)
            nc.tensor.matmul(ps_fin, lhsT=ones_col, rhs=red, start=True, stop=True,
                             skip_group_check=True)
            fin = per.tile([1, 8], f32)
            nc.vector.tensor_copy(fin, ps_fin)
            nc.sync.dma_start(out=out[:, :], in_=fin)

    return nc


_nc_cache = None
last_results = None


def kernel(emissions, tags, mask, transitions, _trace=False):
    global _nc_cache, last_results
    from concourse.bass_utils import run_bass_kernel_spmd
    if _nc_cache is None:
        _nc_cache = _build()
    nc = _nc_cache

    em_flat = np.ascontiguousarray(emissions.reshape(B * S, T).astype(np.float32, copy=False))
    tg_flat = tags.reshape(-1).astype(np.uint16)
    mk_flat = mask.reshape(-1).astype(np.uint16)
    trf = np.ascontiguousarray(transitions.astype(np.float32, copy=False))

    in_maps = []
    for c in range(NCORES):
        lo, hi = c * NPOS, (c + 1) * NPOS
        tg_pad = np.zeros(NPOS + 1, dtype=np.uint16)
        tg_pad[1:] = tg_flat[lo:hi]
        mk_pad = np.zeros(NPOS + 1, dtype=np.uint16)
        mk_pad[1:] = mk_flat[lo:hi]
        in_maps.append({'em': np.ascontiguousarray(em_flat[lo:hi]),
                        'tg': tg_pad, 'mk': mk_pad, 'tr': trf})

    res = run_bass_kernel_spmd(nc, in_maps, core_ids=list(range(NCORES)),
                               trace=_trace)
    last_results = res
    emit = trans = cnt = 0.0
    for r in res.results:
        v = r['out'][0]
        emit += float(v[0])
        trans += float(v[1])
        cnt += float(v[2])
    return np.float32((emit + trans) / cnt)


# revision 2
# speedup vs baseline: 46.8818x; 46.8818x over previous
"""CRF loss kernel for Trainium2 (8 NeuronCores, data-parallel over batch).

Strategy (per core, batch shard of 64 rows = 32768 positions):
  - emissions gather sum_{b,s} m*E[b,s,tags] via one-hot matmuls on PE:
    E is split exactly as E = bf16(E) + bf16(E - bf16(E)) (17-18 mantissa
    bits kept); both halves go through full-rate bf16 matmuls against a
    bf16 one-hot of the (mask-folded) tags, accumulating in fp32 PSUM.
    Diagonal of the accumulated [T,T] PSUM = emission score.
  - transition score via pair co-occurrence counts C = Hprev^T @ Hcur
    (bf16 one-hots, exact 0/1 counts in fp32 PSUM), then sum(C * T).
  - mask folding: tag + 128*(1-m) pushes masked positions out of iota
    range so their one-hot row is all zero.
  - the two scalar partial sums and the mask count are reduced on-chip
    to a [1,8] vector per core; the 8-way combine + division is host-side.
"""
import sys
import json

for p in ('/opt/trn_rl_repo', '/opt/trn_rl_repo/concourse'):
    if p not in sys.path:
        sys.path.insert(0, p)

import numpy as np

B, S, T = 512, 512, 128
NCORES = 8
BSH = B // NCORES              # 64 batch rows per core
NPOS = BSH * S                 # 32768 positions per core
NTILE = NPOS // 128            # 256 tag-tiles of 128 positions
NBLK = NTILE // 4              # 64 blocks of [128, 4, 128]
# fraction of lo-subtract blocks on DVE (rest on GPSIMD)
LO_DVE_MOD = 3                 # g % LO_DVE_MOD == 0 -> DVE


def _split_waits_json(bir_bytes: bytes, max_waits: int = 1) -> bytes:
    """This walrus build accepts at most ONE sync-wait per instruction;
    hoist extra waits onto single-wait NoOps inserted before the inst."""
    d = json.loads(bir_bytes)
    ctr = 0
    for f in d['functions']:
        for blk in f['blocks']:
            insts = blk.get('instructions')
            if not insts:
                continue
            out = []
            changed = False
            for ins in insts:
                si = ins.get('sync_info')
                if si and len(si.get('on_wait') or []) > max_waits:
                    waits = si['on_wait']
                    for w in waits[:-max_waits]:
                        ctr += 1
                        nop = {'engine': ins['engine'], 'ins': [], 'outs': [],
                               'name': f'wsplit-{ctr}', 'opcode': 'NoOp',
                               'sync_info': {'on_wait': [w], 'on_update': []}}
                        if 'debug' in ins:
                            nop['debug'] = ins['debug']
                        out.append(nop)
                    si['on_wait'] = waits[-max_waits:]
                    changed = True
                out.append(ins)
            if changed:
                blk['instructions'] = out
    return json.dumps(d).encode()


_patched = False


def _install_patch(bass_module):
    global _patched
    if _patched:
        return
    _patched = True
    orig = bass_module.Bass.to_json_bytes

    def patched(self):
        return _split_waits_json(orig(self))

    bass_module.Bass.to_json_bytes = patched


def _build():
    import concourse.bass as bass
    import concourse.mybir as mybir
    import concourse.tile as tile
    from concourse.masks import make_identity
    _install_patch(bass)
    f32 = mybir.dt.float32
    bf16 = mybir.dt.bfloat16
    u16 = mybir.dt.uint16
    i32 = mybir.dt.int32
    Alu = mybir.AluOpType

    nc = bass.Bass()
    em = nc.dram_tensor('em', [NPOS, T], f32, kind='ExternalInput')
    tg = nc.dram_tensor('tg', [NPOS + 1], u16, kind='ExternalInput')
    mk = nc.dram_tensor('mk', [NPOS + 1], u16, kind='ExternalInput')
    tr = nc.dram_tensor('tr', [T, T], f32, kind='ExternalInput')
    out = nc.dram_tensor('out', [1, 8], f32, kind='ExternalOutput')

    with tile.TileContext(nc) as tc:
        with tc.tile_pool(name='per', bufs=1) as per, \
             tc.tile_pool(name='eblk', bufs=4) as eblk, \
             tc.tile_pool(name='hblk', bufs=4) as hblk, \
             tc.tile_pool(name='ps', bufs=1, space='PSUM') as psp:

            # ---- constants ----
            iota_i = per.tile([128, 128], i32)
            nc.gpsimd.iota(iota_i, pattern=[[1, 128]], base=0, channel_multiplier=0)
            iota_b = per.tile([128, 128], bf16)
            nc.vector.tensor_copy(iota_b, iota_i)
            ident = per.tile([128, 128], f32)
            make_identity(nc, ident)
            ones_col = per.tile([128, 1], f32)
            nc.vector.memset(ones_col, 1.0)
            t_sb = per.tile([128, 128], f32)
            nc.sync.dma_start(out=t_sb, in_=tr[:, :])

            # ---- tags / mask (transposed to [pos%128, tile] layout) ----
            tg_cur = per.tile([128, NTILE], u16)
            nc.sync.dma_start_transpose(tg_cur, tg[1:NPOS + 1].rearrange("(a b) -> a b", b=128))
            tg_prev = per.tile([128, NTILE], u16)
            nc.sync.dma_start_transpose(tg_prev, tg[0:NPOS].rearrange("(a b) -> a b", b=128))
            mk_cur = per.tile([128, NTILE], u16)
            nc.sync.dma_start_transpose(mk_cur, mk[1:NPOS + 1].rearrange("(a b) -> a b", b=128))
            mk_prev = per.tile([128, NTILE], u16)
            nc.sync.dma_start_transpose(mk_prev, mk[0:NPOS].rearrange("(a b) -> a b", b=128))

            tgc_f = per.tile([128, NTILE], f32)
            nc.vector.tensor_copy(tgc_f, tg_cur)
            tgp_f = per.tile([128, NTILE], f32)
            nc.vector.tensor_copy(tgp_f, tg_prev)
            mc_f = per.tile([128, NTILE], f32)
            nc.vector.tensor_copy(mc_f, mk_cur)
            mp_f = per.tile([128, NTILE], f32)
            nc.vector.tensor_copy(mp_f, mk_prev)

            # masked cur tags: tg + 128 - 128*m
            tmp = per.tile([128, NTILE], f32)
            nc.vector.tensor_scalar(out=tmp, in0=mc_f, scalar1=-128.0, scalar2=128.0,
                                    op0=Alu.mult, op1=Alu.add)
            mtag_c = per.tile([128, NTILE], f32)
            nc.vector.tensor_add(mtag_c, tgc_f, tmp)

            # pair mask pm = m_cur * m_prev, zeroed at batch-row starts
            pm = per.tile([128, NTILE], f32)
            nc.vector.tensor_mul(pm, mc_f, mp_f)
            rs_i = per.tile([128, NTILE], i32)   # p + 128*(tile%4); ==0 at row starts
            nc.gpsimd.iota(rs_i, pattern=[[0, NTILE // 4], [128, 4]], base=0,
                           channel_multiplier=1)
            rs_f = per.tile([128, NTILE], f32)
            nc.vector.tensor_copy(rs_f, rs_i)
            rs_m = per.tile([128, NTILE], f32)
            nc.vector.tensor_scalar(out=rs_m, in0=rs_f, scalar1=0.0, scalar2=None,
                                    op0=Alu.not_equal)
            nc.vector.tensor_mul(pm, pm, rs_m)

            # masked prev tags: tg_prev + 128 - 128*pm
            nc.vector.tensor_scalar(out=tmp, in0=pm, scalar1=-128.0, scalar2=128.0,
                                    op0=Alu.mult, op1=Alu.add)
            mtag_p = per.tile([128, NTILE], f32)
            nc.vector.tensor_add(mtag_p, tgp_f, tmp)

            # ---- accumulators ----
            ps_emit = psp.tile([128, 256], f32)
            ps_c = psp.tile([128, 128], f32)

            em_r = em.rearrange("(g j p) t -> g p j t", p=128, j=4)

            for g in range(NBLK):
                e_blk = eblk.tile([128, 4, 128], f32, tag='e')
                nc.sync.dma_start(out=e_blk, in_=em_r[g])
                hl_blk = eblk.tile([128, 4, 256], bf16, tag='hl')
                hi_blk = hl_blk[:, :, 0:128]
                lo_blk = hl_blk[:, :, 128:256]
                nc.scalar.activation(out=hi_blk, in_=e_blk,
                                     func=mybir.ActivationFunctionType.Copy)
                if g % LO_DVE_MOD == 0:
                    nc.vector.tensor_sub(lo_blk, e_blk, hi_blk)
                else:
                    nc.gpsimd.tensor_sub(lo_blk, e_blk, hi_blk)
                hm = hblk.tile([128, 4, 128], bf16, tag='hm')
                hp = hblk.tile([128, 4, 128], bf16, tag='hp')
                for j in range(4):
                    k = 4 * g + j
                    nc.vector.tensor_scalar(out=hm[:, j, :], in0=iota_b,
                                            scalar1=mtag_c[:, k:k + 1], scalar2=None,
                                            op0=Alu.is_equal)
                    nc.vector.tensor_scalar(out=hp[:, j, :], in0=iota_b,
                                            scalar1=mtag_p[:, k:k + 1], scalar2=None,
                                            op0=Alu.is_equal)
                for j in range(4):
                    first = (g == 0 and j == 0)
                    last = (g == NBLK - 1 and j == 3)
                    nc.tensor.matmul(ps_emit, lhsT=hm[:, j, :], rhs=hl_blk[:, j, :],
                                     start=first, stop=last, skip_group_check=True)
                    nc.tensor.matmul(ps_c, lhsT=hp[:, j, :], rhs=hm[:, j, :],
                                     start=first, stop=last, skip_group_check=True)

            # ---- final reductions ----
            red = per.tile([128, 8], f32)
            nc.vector.memset(red, 0.0)
            scr = per.tile([128, 256], f32)
            nc.vector.tensor_mul(scr[:, 0:128], ps_emit[:, 0:128], ident)
            nc.vector.tensor_mul(scr[:, 128:256], ps_emit[:, 128:256], ident)
            nc.vector.tensor_reduce(out=red[:, 0:1], in_=scr,
                                    axis=mybir.AxisListType.X, op=Alu.add)
            scr2 = per.tile([128, 128], f32)
            nc.vector.tensor_mul(scr2, ps_c, t_sb)
            nc.vector.tensor_reduce(out=red[:, 1:2], in_=scr2,
                                    axis=mybir.AxisListType.X, op=Alu.add)
            nc.vector.tensor_reduce(out=red[:, 2:3], in_=mc_f,
                                    axis=mybir.AxisListType.X, op=Alu.add)
            ps_fin = psp.tile([1, 8], f32)
            nc.tensor.matmul(ps_fin, lhsT=ones_col, rhs=red, start=True, stop=True,
                             skip_group_check=True)
            fin = per.tile([1, 8], f32)
            nc.vector.tensor_copy(fin, ps_fin)
            nc.sync.dma_start(out=out[:, :], in_=fin)

    return nc


_nc_cache = None
last_results = None


def kernel(emissions, tags, mask, transitions, _trace=False):
    global _nc_cache, last_results
    from concourse.bass_utils import run_bass_kernel_spmd
    if _nc_cache is None:
        _nc_cache = _build()
    nc = _nc_cache

    em_flat = np.ascontiguousarray(emissions.reshape(B * S, T).astype(np.float32, copy=False))
    tg_flat = tags.reshape(-1).astype(np.uint16)
    mk_flat = mask.reshape(-1).astype(np.uint16)
    trf = np.ascontiguousarray(transitions.astype(np.float32, copy=False))

    in_maps = []
    for c in range(NCORES):
        lo, hi = c * NPOS, (c + 1) * NPOS
        tg_pad = np.zeros(NPOS + 1, dtype=np.uint16)
        tg_pad[1:] = tg_flat[lo:hi]
        mk_pad = np.zeros(NPOS + 1, dtype=np.uint16)
        mk_pad[1:] = mk_flat[lo:hi]
        in_maps.append({'em': np.ascontiguousarray(em_flat[lo:hi]),
                        'tg': tg_pad, 'mk': mk_pad, 'tr': trf})

    res = run_bass_kernel_spmd(nc, in_maps, core_ids=list(range(NCORES)),
                               trace=_trace)
    last_results = res
    emit = trans = cnt = 0.0
    for r in res.results:
        v = r['out'][0]
        emit += float(v[0])
        trans += float(v[1])
        cnt += float(v[2])
    return np.float32((emit + trans) / cnt)
